# revision 26
# baseline (speedup 1.0000x reference)
"""HalfKP input layer (embedding_lookup) on 8 Trainium2 NeuronCores.

Reference computation (B=1024, K=64, F=640, C=256):
    p = piece_positions.reshape(B, 640).astype(f32)          # values in {0,1}
    Wg = input_weights[king_positions]                       # (B, 2, 641, 256)
    out[b] = sum_f p[b,f] * (Wg[b,0,f,:] + Wg[b,1,f,:])
             + Wg[b,0,640,:] + Wg[b,1,640,:] + bias

Strategy — king-sharded so the 21MB bf16 table is read exactly once in
aggregate (~2.6MB/core, the HBM roofline ~9us at ~310GB/s/core):
  * The 2048 (sample, king-slot) pairs are grouped by king square on the
    host; king squares are distributed over the 8 cores balanced by row
    count, S slots per core, each slot padded to G rows.
  * Weights stream as a single bf16 copy; features (0/1, exact in fp8e4)
    are the stationary operand; two G=64 slots are packed per
    128-partition PSUM tile.  Queue schedule (empirically fastest; both
    coarser fusing and pack-eager reordering regressed):
      sync  : warmup, feats(fp8), s0, s2, s4, rows0, rows2
      scalar: warmup, s1, s3, s5, s6, s7, rows1, rows3
      gpsimd: valid2, wex2 (tiny)
  * Row 640 of both slots (+ bias/2, folded host-side) lands via ONE K=2
    matmul per pack gated by the valid mask.
  * Tiny warm-up DMAs touch all 16 DMA engines first: engine 15 pays a
    ~3us one-time init on its first descriptor, which otherwise gates the
    first completion semaphore (it delayed launch 2's add by ~3us).
  * Launch 1 emits (S*G, 256) pair rows fp16; the host routes rows to the
    batch-owning cores (pure indexing).  Launch 2: out[b] = rowA + rowB.

Measured notes (NTFF traces): per-launch fixed cost is ~13.5us of
exec_time_ns even for a trivial kernel (~5.6us prologue excluded, ~8.4us
postamble of 255 serialized semaphore clears included); HWDGE queues lose
~1us per extra trigger, so DMA count is kept minimal; collectives were
~60us end-to-end, so host routing between two launches wins.  Best
measured total: ~41.0us (launch 1 ~25.6us, launch 2 ~15.5us) vs the
53.7us session baseline.
"""

import os
from contextlib import ExitStack

import numpy as np
import ml_dtypes

import concourse.bass as bass
import concourse.tile as tile
from concourse import bacc, mybir
from concourse.bass_utils import run_bass_kernel_spmd

B = 1024
K = 64
F = 640
C = 256
NCORES = 8
FCH = F // 128  # 5 feature chunks of 128
P = 128

BF16 = ml_dtypes.bfloat16
FP16 = np.float16

# Exposed for test harnesses
LAST_RESULTS = []
LAST_EXEC_NS = None

_cache = {}


def _build_main(S: int, G: int):
    """Launch-1 program: per-king-slot matmuls -> pair rows (S*G, C) fp16."""
    PK = P // G  # slots per 128-partition pack
    NPK = S // PK
    nc = bacc.Bacc(
        "TRN2", target_bir_lowering=False, debug=False, num_devices=NCORES
    )
    dt = mybir.dt

    # w_in[r, j, ch, :] = bf16(W[k_j, ch*128+r, :])
    w_in = nc.dram_tensor("w_in", [P, S, FCH, C], dt.bfloat16, kind="ExternalInput")
    feats = nc.dram_tensor("feats", [P, S, FCH, G], dt.float8e4, kind="ExternalInput")
    # valid2[h, pk, 0:P] = valid mask of pack pk's slot h, offset h*G
    valid2 = nc.dram_tensor("valid2", [2, NPK, P], dt.bfloat16, kind="ExternalInput")
    # wex2[h, pk, :] = bf16(W[k of slot 2*pk+h, 640, :] + bias/2)
    wex2 = nc.dram_tensor("wex2", [2, NPK, C], dt.bfloat16, kind="ExternalInput")
    rows_out = nc.dram_tensor("rows_out", [S * G, C], dt.float16, kind="ExternalOutput")

    with tile.TileContext(nc) as tc, ExitStack() as ctx:
        const_pool = ctx.enter_context(tc.tile_pool(name="const", bufs=1))
        w_pool = ctx.enter_context(tc.tile_pool(name="w", bufs=8))
        rows_pool = ctx.enter_context(tc.tile_pool(name="rows", bufs=4))
        psum_pool = ctx.enter_context(tc.tile_pool(name="psum", bufs=4, space="PSUM"))

        # Queue schedule — empirically the fastest found; both coarser
        # trigger fusing and pack-eager reordering regressed:
        #   sync  : warmup, feats, s0, s2, s4, rows0, rows2
        #   scalar: warmup, s1, s3, s5, s6, s7, rows1, rows3
        #   gpsimd: valid2, wex2 (tiny)
        wm0 = const_pool.tile([P, 8], dt.float8e4)
        nc.sync.dma_start(out=wm0[:], in_=feats[:, 0, 0, 0:8])
        wm1 = const_pool.tile([P, 8], dt.float8e4)
        nc.scalar.dma_start(out=wm1[:], in_=feats[:, 0, 0, 0:8])

        feats_sb = const_pool.tile([P, S * FCH * G], dt.float8e4)
        nc.sync.dma_start(
            out=feats_sb[:], in_=feats.ap().rearrange("p s ch g -> p (s ch g)")
        )

        # per-slot weight slabs over the two HWDGE queues, byte-balanced
        # against the feats DMA riding the sync queue
        w_slot = []
        for j in range(S):
            w_sb = w_pool.tile([P, FCH * C], dt.bfloat16, tag="w")
            eng = nc.sync if j in (0, 2, 4) else nc.scalar
            eng.dma_start(
                out=w_sb[:],
                in_=w_in[:, j, :, :].rearrange("p ch c -> p (ch c)"),
            )
            w_slot.append(w_sb)

        valid_sb = const_pool.tile([2, NPK * P], dt.bfloat16)
        nc.gpsimd.dma_start(
            out=valid_sb[:], in_=valid2.ap().rearrange("h pk p -> h (pk p)")
        )
        wex_sb = const_pool.tile([2, NPK * C], dt.bfloat16)
        nc.gpsimd.dma_start(
            out=wex_sb[:], in_=wex2.ap().rearrange("h pk c -> h (pk c)")
        )

        def feats_ap(pk, j2, ch):
            base = ((pk * PK + j2) * FCH + ch) * G
            return feats_sb[:, base : base + G]

        for pk in range(NPK):
            acc = psum_pool.tile([P, C], dt.float32, space="PSUM")
            for ch in range(FCH):
                for j2 in range(PK):
                    j = pk * PK + j2
                    nc.tensor.matmul(
                        out=acc[j2 * G : (j2 + 1) * G, :],
                        lhsT=feats_ap(pk, j2, ch),
                        rhs=w_slot[j][:, ch * C : (ch + 1) * C],
                        start=(ch == 0),
                        stop=False,
                    )
            # row 640 of both slots (+bias/2), one K=2 matmul per pack
            nc.tensor.matmul(
                out=acc[:, :],
                lhsT=valid_sb[0:2, pk * P : (pk + 1) * P],
                rhs=wex_sb[0:2, pk * C : (pk + 1) * C],
                start=False,
                stop=True,
            )
            rows_sb = rows_pool.tile([P, C], dt.float16, tag="rows")
            nc.vector.tensor_copy(rows_sb[:, :], acc[:, :])
            (nc.scalar if pk % 2 else nc.sync).dma_start(
                out=rows_out[pk * P : (pk + 1) * P, :], in_=rows_sb[:, :]
            )

    nc.compile()
    return nc


def _build_final():
    """Launch-2 program: out[b] = rowA(b) + rowB(b)."""
    nc = bacc.Bacc(
        "TRN2", target_bir_lowering=False, debug=False, num_devices=NCORES
    )
    dt = mybir.dt
    fin_in = nc.dram_tensor("fin_in", [P, 2, C], dt.float16, kind="ExternalInput")
    out = nc.dram_tensor("out", [P, C], dt.float32, kind="ExternalOutput")

    with tile.TileContext(nc) as tc, ExitStack() as ctx:
        pool = ctx.enter_context(tc.tile_pool(name="sbuf", bufs=1))
        # warm all 16 DMA engines (engine 15 pays ~3us init on its first
        # descriptor; nothing waits on this tile)
        wm0 = pool.tile([P, 8], dt.float16)
        nc.sync.dma_start(out=wm0[:], in_=fin_in[:, 0, 0:8])
        wm1 = pool.tile([P, 8], dt.float16)
        nc.scalar.dma_start(out=wm1[:], in_=fin_in[:, 0, 0:8])

        t = pool.tile([P, 2 * C], dt.float16)
        nc.sync.dma_start(out=t[:], in_=fin_in.ap().rearrange("p t c -> p (t c)"))
        s1 = pool.tile([P, C], dt.float32)
        nc.vector.tensor_add(s1[:], t[:, 0:C], t[:, C : 2 * C])
        # output rides the otherwise-idle scalar queue: its trigger is
        # pre-fetched and fires the moment the add's semaphore lands,
        # instead of queuing behind the input DMA on sync
        nc.scalar.dma_start(out=out[:, :], in_=s1[:])

    nc.compile()
    return nc


def _shard(king_positions):
    """Group the 2048 (sample, s) pairs by king square, balance over cores."""
    kings = np.asarray(king_positions).astype(np.int64)  # (B, 2)

    groups = [[] for _ in range(K)]
    for b in range(B):
        groups[kings[b, 0]].append((b, 0))
        groups[kings[b, 1]].append((b, 1))

    max_group = max(len(g) for g in groups)
    G = 64 if max_group <= 64 else 128
    chunks = []  # (king, rows) with <= G rows each
    for k in range(K):
        g = groups[k]
        for i in range(0, max(len(g), 1), G):
            chunks.append((k, g[i : i + G]))

    PK = P // G
    S = -(-len(chunks) // NCORES)
    S = -(-S // PK) * PK  # packs tile evenly
    chunks.sort(key=lambda c: -len(c[1]))
    core_chunks = [[] for _ in range(NCORES)]
    core_rows = [0] * NCORES
    for chk in chunks:
        cands = [c for c in range(NCORES) if len(core_chunks[c]) < S]
        c = min(cands, key=lambda c: core_rows[c])
        core_chunks[c].append(chk)
        core_rows[c] += len(chk[1])
    for c in range(NCORES):
        while len(core_chunks[c]) < S:
            core_chunks[c].append((0, []))
    return core_chunks, S, G


def kernel(piece_positions, king_positions, input_weights, bias):
    global LAST_RESULTS, LAST_EXEC_NS

    p_flat = np.asarray(piece_positions).reshape(B, F).astype(np.float32)
    w_full = np.ascontiguousarray(np.asarray(input_weights), dtype=np.float32)
    bias_np = np.asarray(bias, dtype=np.float32)

    core_chunks, S, G = _shard(king_positions)
    PK = P // G
    NPK = S // PK

    if ("main", S, G) not in _cache:
        _cache[("main", S, G)] = _build_main(S, G)
    if "final" not in _cache:
        _cache["final"] = _build_final()
    nc_main = _cache[("main", S, G)]
    nc_final = _cache["final"]

    w_bf = w_full.astype(BF16)
    wex_full = (w_full[:, F, :] + 0.5 * bias_np).astype(BF16)  # (K, C)

    pair_row = np.zeros((B, 2), dtype=np.int64)
    in_maps = []
    for c in range(NCORES):
        kc = np.array([k for k, _ in core_chunks[c]], dtype=np.int64)  # (S,)
        # (S, 640, C) -> (P, S, FCH, C)
        whl = w_bf[kc][:, :F, :].reshape(S, FCH, 128, C).transpose(2, 0, 1, 3)

        wexc = np.zeros((2, NPK, C), dtype=np.float32)
        ft = np.zeros((S, G, FCH, 128), dtype=np.float32)
        vl = np.zeros((2, NPK, P), dtype=np.float32)
        for j, (k, rows) in enumerate(core_chunks[c]):
            pk, j2 = divmod(j, PK)
            wexc[j2, pk, :] = wex_full[k]
            n = len(rows)
            if n:
                bs = np.array([b for b, _ in rows], dtype=np.int64)
                ft[j, :n] = p_flat[bs].reshape(n, FCH, 128)
                vl[j2, pk, j2 * G : j2 * G + n] = 1.0
                for i, (b, s) in enumerate(rows):
                    pair_row[b, s] = c * S * G + j * G + i
        ftT = ft.transpose(3, 0, 2, 1)  # (128, S, FCH, G)

        in_maps.append(
            {
                "w_in": np.ascontiguousarray(whl),
                "feats": np.ascontiguousarray(ftT).astype(ml_dtypes.float8_e4m3),
                "valid2": np.ascontiguousarray(vl).astype(BF16),
                "wex2": np.ascontiguousarray(wexc).astype(BF16),
            }
        )

    do_trace = bool(int(os.environ.get("KERNEL_TRACE", "0")))
    trace_kw = dict(
        trace=do_trace, trace_cores=list(range(NCORES)) if do_trace else None
    )

    res1 = run_bass_kernel_spmd(nc_main, in_maps, list(range(NCORES)), **trace_kw)

    # host routing: pure indexing, no arithmetic
    rows_all = np.concatenate(
        [res1.results[c]["rows_out"] for c in range(NCORES)], axis=0
    )
    in_maps2 = []
    for c in range(NCORES):
        fin = np.empty((P, 2, C), dtype=FP16)
        sl = pair_row[c * P : (c + 1) * P]  # (128, 2)
        fin[:, 0, :] = rows_all[sl[:, 0]]
        fin[:, 1, :] = rows_all[sl[:, 1]]
        in_maps2.append({"fin_in": fin})
    res2 = run_bass_kernel_spmd(nc_final, in_maps2, list(range(NCORES)), **trace_kw)

    LAST_RESULTS = [res1, res2]
    if res1.exec_time_ns is not None and res2.exec_time_ns is not None:
        LAST_EXEC_NS = res1.exec_time_ns + res2.exec_time_ns
    else:
        LAST_EXEC_NS = None

    outs = [res2.results[c]["out"] for c in range(NCORES)]
    return np.ascontiguousarray(np.concatenate(outs, axis=0))


# revision 27
# speedup vs baseline: 1.0599x; 1.0599x over previous
"""HalfKP input layer (embedding_lookup) on 8 Trainium2 NeuronCores.

Reference computation (B=1024, K=64, F=640, C=256):
    p = piece_positions.reshape(B, 640).astype(f32)          # values in {0,1}
    Wg = input_weights[king_positions]                       # (B, 2, 641, 256)
    out[b] = sum_f p[b,f] * (Wg[b,0,f,:] + Wg[b,1,f,:])
             + Wg[b,0,640,:] + Wg[b,1,640,:] + bias

Strategy — king-sharded so the 21MB bf16 table is read exactly once in
aggregate (~2.6MB/core, the HBM roofline ~9us at ~310GB/s/core):
  * The 2048 (sample, king-slot) pairs are grouped by king square on the
    host; king squares are distributed over the 8 cores balanced by row
    count, S slots per core, each slot padded to G rows.
  * Weights stream as a single bf16 copy; features (0/1, exact in fp8e4)
    are the stationary operand; two G=64 slots are packed per
    128-partition PSUM tile.  Queue schedule (empirically fastest; both
    coarser fusing and pack-eager reordering regressed):
      sync  : warmup, feats(fp8), s0, s2, s4, rows0, rows2
      scalar: warmup, s1, s3, s5, s6, s7, rows1, rows3
      gpsimd: valid2, wex2 (tiny)
  * Row 640 of both slots (+ bias/2, folded host-side) lands via ONE K=2
    matmul per pack gated by the valid mask.
  * Tiny warm-up DMAs touch all 16 DMA engines first: engine 15 pays a
    ~3us one-time init on its first descriptor, which otherwise gates the
    first completion semaphore (it delayed launch 2's add by ~3us).
  * Launch 1 emits (S*G, 256) pair rows fp16; the host routes rows to the
    batch-owning cores (pure indexing).  Launch 2: out[b] = rowA + rowB.

Measured notes (NTFF traces): per-launch fixed cost is ~13.5us of
exec_time_ns even for a trivial kernel (~5.6us prologue excluded, ~8.4us
postamble of 255 serialized semaphore clears included); HWDGE queues lose
~1us per extra trigger, so DMA count is kept minimal; collectives were
~60us end-to-end, so host routing between two launches wins.  Best
measured total: ~41.0us (launch 1 ~25.6us, launch 2 ~15.5us) vs the
53.7us session baseline.
"""

import os
from contextlib import ExitStack

import numpy as np
import ml_dtypes

import concourse.bass as bass
import concourse.tile as tile
from concourse import bacc, mybir
from concourse.bass_utils import run_bass_kernel_spmd

B = 1024
K = 64
F = 640
C = 256
NCORES = 8
FCH = F // 128  # 5 feature chunks of 128
P = 128

BF16 = ml_dtypes.bfloat16
FP16 = np.float16

# Exposed for test harnesses
LAST_RESULTS = []
LAST_EXEC_NS = None

_cache = {}


def _build_main(S: int, G: int):
    """Launch-1 program: per-king-slot matmuls -> pair rows (S*G, C) fp16."""
    PK = P // G  # slots per 128-partition pack
    NPK = S // PK
    nc = bacc.Bacc(
        "TRN2", target_bir_lowering=False, debug=False, num_devices=NCORES
    )
    dt = mybir.dt

    # w_in[r, j, ch, :] = bf16(W[k_j, ch*128+r, :])
    w_in = nc.dram_tensor("w_in", [P, S, FCH, C], dt.bfloat16, kind="ExternalInput")
    feats = nc.dram_tensor("feats", [P, S, FCH, G], dt.float8e4, kind="ExternalInput")
    # valid2[h, pk, 0:P] = valid mask of pack pk's slot h, offset h*G
    valid2 = nc.dram_tensor("valid2", [2, NPK, P], dt.bfloat16, kind="ExternalInput")
    # wex2[h, pk, :] = bf16(W[k of slot 2*pk+h, 640, :] + bias/2)
    wex2 = nc.dram_tensor("wex2", [2, NPK, C], dt.bfloat16, kind="ExternalInput")
    rows_out = nc.dram_tensor("rows_out", [S * G, C], dt.float16, kind="ExternalOutput")

    with tile.TileContext(nc) as tc, ExitStack() as ctx:
        const_pool = ctx.enter_context(tc.tile_pool(name="const", bufs=1))
        w_pool = ctx.enter_context(tc.tile_pool(name="w", bufs=8))
        rows_pool = ctx.enter_context(tc.tile_pool(name="rows", bufs=4))
        psum_pool = ctx.enter_context(tc.tile_pool(name="psum", bufs=4, space="PSUM"))

        # Queue schedule — empirically the fastest found; both coarser
        # trigger fusing and pack-eager reordering regressed:
        #   sync  : warmup, feats, s0, s2, s4, rows0, rows2
        #   scalar: warmup, s1, s3, s5, s6, s7, rows1, rows3
        #   gpsimd: valid2, wex2 (tiny)
        wm0 = const_pool.tile([P, 8], dt.float8e4)
        nc.sync.dma_start(out=wm0[:], in_=feats[:, 0, 0, 0:8])
        wm1 = const_pool.tile([P, 8], dt.float8e4)
        nc.scalar.dma_start(out=wm1[:], in_=feats[:, 0, 0, 0:8])

        feats_sb = const_pool.tile([P, S * FCH * G], dt.float8e4)
        nc.sync.dma_start(
            out=feats_sb[:], in_=feats.ap().rearrange("p s ch g -> p (s ch g)")
        )

        # per-slot weight slabs over the two HWDGE queues, byte-balanced
        # against the feats DMA riding the sync queue
        w_slot = []
        for j in range(S):
            w_sb = w_pool.tile([P, FCH * C], dt.bfloat16, tag="w")
            eng = nc.sync if j in (0, 2, 4) else nc.scalar
            eng.dma_start(
                out=w_sb[:],
                in_=w_in[:, j, :, :].rearrange("p ch c -> p (ch c)"),
            )
            w_slot.append(w_sb)

        valid_sb = const_pool.tile([2, NPK * P], dt.bfloat16)
        nc.gpsimd.dma_start(
            out=valid_sb[:], in_=valid2.ap().rearrange("h pk p -> h (pk p)")
        )
        wex_sb = const_pool.tile([2, NPK * C], dt.bfloat16)
        nc.gpsimd.dma_start(
            out=wex_sb[:], in_=wex2.ap().rearrange("h pk c -> h (pk c)")
        )

        def feats_ap(pk, j2, ch):
            base = ((pk * PK + j2) * FCH + ch) * G
            return feats_sb[:, base : base + G]

        for pk in range(NPK):
            acc = psum_pool.tile([P, C], dt.float32, space="PSUM")
            for ch in range(FCH):
                for j2 in range(PK):
                    j = pk * PK + j2
                    nc.tensor.matmul(
                        out=acc[j2 * G : (j2 + 1) * G, :],
                        lhsT=feats_ap(pk, j2, ch),
                        rhs=w_slot[j][:, ch * C : (ch + 1) * C],
                        start=(ch == 0),
                        stop=False,
                    )
            # row 640 of both slots (+bias/2), one K=2 matmul per pack
            nc.tensor.matmul(
                out=acc[:, :],
                lhsT=valid_sb[0:2, pk * P : (pk + 1) * P],
                rhs=wex_sb[0:2, pk * C : (pk + 1) * C],
                start=False,
                stop=True,
            )
            rows_sb = rows_pool.tile([P, C], dt.float16, tag="rows")
            nc.vector.tensor_copy(rows_sb[:, :], acc[:, :])
            (nc.scalar if pk % 2 else nc.sync).dma_start(
                out=rows_out[pk * P : (pk + 1) * P, :], in_=rows_sb[:, :]
            )

    nc.compile()
    return nc


def _build_final():
    """Launch-2 program: out[b] = rowA(b) + rowB(b)."""
    nc = bacc.Bacc(
        "TRN2", target_bir_lowering=False, debug=False, num_devices=NCORES
    )
    dt = mybir.dt
    fin_in = nc.dram_tensor("fin_in", [P, 2, C], dt.float16, kind="ExternalInput")
    out = nc.dram_tensor("out", [P, C], dt.float32, kind="ExternalOutput")

    with tile.TileContext(nc) as tc, ExitStack() as ctx:
        pool = ctx.enter_context(tc.tile_pool(name="sbuf", bufs=1))
        # warm all 16 DMA engines (engine 15 pays ~3us init on its first
        # descriptor; nothing waits on this tile)
        wm0 = pool.tile([P, 8], dt.float16)
        nc.sync.dma_start(out=wm0[:], in_=fin_in[:, 0, 0:8])
        t = pool.tile([P, 2 * C], dt.float16)
        nc.sync.dma_start(out=t[:], in_=fin_in.ap().rearrange("p t c -> p (t c)"))
        s1 = pool.tile([P, C], dt.float32)
        nc.vector.tensor_add(s1[:], t[:, 0:C], t[:, C : 2 * C])
        nc.sync.dma_start(out=out[:, :], in_=s1[:])

    nc.compile()
    return nc


def _shard(king_positions):
    """Group the 2048 (sample, s) pairs by king square, balance over cores."""
    kings = np.asarray(king_positions).astype(np.int64)  # (B, 2)

    groups = [[] for _ in range(K)]
    for b in range(B):
        groups[kings[b, 0]].append((b, 0))
        groups[kings[b, 1]].append((b, 1))

    max_group = max(len(g) for g in groups)
    G = 64 if max_group <= 64 else 128
    chunks = []  # (king, rows) with <= G rows each
    for k in range(K):
        g = groups[k]
        for i in range(0, max(len(g), 1), G):
            chunks.append((k, g[i : i + G]))

    PK = P // G
    S = -(-len(chunks) // NCORES)
    S = -(-S // PK) * PK  # packs tile evenly
    chunks.sort(key=lambda c: -len(c[1]))
    core_chunks = [[] for _ in range(NCORES)]
    core_rows = [0] * NCORES
    for chk in chunks:
        cands = [c for c in range(NCORES) if len(core_chunks[c]) < S]
        c = min(cands, key=lambda c: core_rows[c])
        core_chunks[c].append(chk)
        core_rows[c] += len(chk[1])
    for c in range(NCORES):
        while len(core_chunks[c]) < S:
            core_chunks[c].append((0, []))
    return core_chunks, S, G


def kernel(piece_positions, king_positions, input_weights, bias):
    global LAST_RESULTS, LAST_EXEC_NS

    p_flat = np.asarray(piece_positions).reshape(B, F).astype(np.float32)
    w_full = np.ascontiguousarray(np.asarray(input_weights), dtype=np.float32)
    bias_np = np.asarray(bias, dtype=np.float32)

    core_chunks, S, G = _shard(king_positions)
    PK = P // G
    NPK = S // PK

    if ("main", S, G) not in _cache:
        _cache[("main", S, G)] = _build_main(S, G)
    if "final" not in _cache:
        _cache["final"] = _build_final()
    nc_main = _cache[("main", S, G)]
    nc_final = _cache["final"]

    w_bf = w_full.astype(BF16)
    wex_full = (w_full[:, F, :] + 0.5 * bias_np).astype(BF16)  # (K, C)

    pair_row = np.zeros((B, 2), dtype=np.int64)
    in_maps = []
    for c in range(NCORES):
        kc = np.array([k for k, _ in core_chunks[c]], dtype=np.int64)  # (S,)
        # (S, 640, C) -> (P, S, FCH, C)
        whl = w_bf[kc][:, :F, :].reshape(S, FCH, 128, C).transpose(2, 0, 1, 3)

        wexc = np.zeros((2, NPK, C), dtype=np.float32)
        ft = np.zeros((S, G, FCH, 128), dtype=np.float32)
        vl = np.zeros((2, NPK, P), dtype=np.float32)
        for j, (k, rows) in enumerate(core_chunks[c]):
            pk, j2 = divmod(j, PK)
            wexc[j2, pk, :] = wex_full[k]
            n = len(rows)
            if n:
                bs = np.array([b for b, _ in rows], dtype=np.int64)
                ft[j, :n] = p_flat[bs].reshape(n, FCH, 128)
                vl[j2, pk, j2 * G : j2 * G + n] = 1.0
                for i, (b, s) in enumerate(rows):
                    pair_row[b, s] = c * S * G + j * G + i
        ftT = ft.transpose(3, 0, 2, 1)  # (128, S, FCH, G)

        in_maps.append(
            {
                "w_in": np.ascontiguousarray(whl),
                "feats": np.ascontiguousarray(ftT).astype(ml_dtypes.float8_e4m3),
                "valid2": np.ascontiguousarray(vl).astype(BF16),
                "wex2": np.ascontiguousarray(wexc).astype(BF16),
            }
        )

    do_trace = bool(int(os.environ.get("KERNEL_TRACE", "0")))
    trace_kw = dict(
        trace=do_trace, trace_cores=list(range(NCORES)) if do_trace else None
    )

    res1 = run_bass_kernel_spmd(nc_main, in_maps, list(range(NCORES)), **trace_kw)

    # host routing: pure indexing, no arithmetic
    rows_all = np.concatenate(
        [res1.results[c]["rows_out"] for c in range(NCORES)], axis=0
    )
    in_maps2 = []
    for c in range(NCORES):
        fin = np.empty((P, 2, C), dtype=FP16)
        sl = pair_row[c * P : (c + 1) * P]  # (128, 2)
        fin[:, 0, :] = rows_all[sl[:, 0]]
        fin[:, 1, :] = rows_all[sl[:, 1]]
        in_maps2.append({"fin_in": fin})
    res2 = run_bass_kernel_spmd(nc_final, in_maps2, list(range(NCORES)), **trace_kw)

    LAST_RESULTS = [res1, res2]
    if res1.exec_time_ns is not None and res2.exec_time_ns is not None:
        LAST_EXEC_NS = res1.exec_time_ns + res2.exec_time_ns
    else:
        LAST_EXEC_NS = None

    outs = [res2.results[c]["out"] for c in range(NCORES)]
    return np.ascontiguousarray(np.concatenate(outs, axis=0))
